# revision 3
# baseline (speedup 1.0000x reference)
"""Trainium2 Bass kernel for nn_AxonalConnections (gnn_message_passing).

Computes, for 4 modules with 12 directed pairs (s, d), s != d:
    out[d] = sum_{s != d} x[s] @ W[(s,d)].T
             + strength[d] * (sin(t*local_freq[d]) + sin(t*global_freq[d]))
with x: [4, 2048, 1024] f32, W: [12, 1024, 1024] f32, t = 2*pi*clk*1e-3.

Sharding over 8 NeuronCores: core c = 2*d + h handles destination module d
and batch half h (1024 rows).  Per core: 3 GEMMs [1024,1024]@[1024,1024]
accumulated in PSUM.  The oscillator bias row is broadcast to a [128,D]
SBUF tile once (ones-matmul) and added by the vector engine during the
PSUM->SBUF drain, so the PE only runs the 384 real matmuls.

Perf notes (v2):
- GEMM operands use float32r (TF32-class): 1 cycle/row on the PE for
  N>=256 — the PE stream floor is 384 x 512 rows ~ 82 us at 2.4 GHz and
  the kernel is PE-bound, so everything else is scheduled around keeping
  the PE stream gapless.
- DMA kick cost is ~0.6 us per dma_start on the issuing engine's queue:
  W tiles are fetched as single [128,1024] 512 KB kicks (24 instead of
  48) so the input stream is no longer kick-rate-bound (~380 GB/s in v1).
- Output drains go through the *scalar* engine's DMA queue; input loads
  own the sync queue.  This kills the head-of-line blocking that used to
  delay all 4 MiB of output writes to a serial tail after the last
  matmul.
- Batch group 1 computes its last two row-tiles (bi=6,7) with the
  (j,k)-sweep innermost and drains each row-tile immediately, so the
  final output DMA overlaps compute and the post-matmul tail is ~2 us.
- Dummy warm-up matmuls right after the entry barrier hold the PE's HAM
  activity monitor busy so the clock ramps to 2.4 GHz as early as
  possible.
- The Bass program is built by code exec'd under a fixed pseudo-filename
  so the BIR (which embeds source debug locations) is byte-identical no
  matter where kernel.py lives — keeping the NEFF compile cache warm
  across directories.

Host-side prep is limited to slicing/transposing inputs into the per-core
layouts (contraction dim on partitions) and computing the scalar t.
"""

import math
import sys
import threading

import numpy as np

sys.path.insert(0, "/opt/trn_rl_repo")

from concourse.bass_utils import run_bass_kernel_spmd  # noqa: E402

N_MOD = 4
B = 2048
D = 1024
BH = B // 2  # batch rows per core
N_CORES = 8

PAIRS = [(s, d) for s in range(N_MOD) for d in range(N_MOD) if s != d]
PAIR_IDX = {sd: i for i, sd in enumerate(PAIRS)}
SRCS_OF = {d: [s for s in range(N_MOD) if s != d] for d in range(N_MOD)}

_CACHED = {}

_BUILDER_FILENAME = "/bass_axonal_connections/builder.py"
_BUILDER_SRC = '''
import concourse.mybir as mybir
from concourse import bacc
from concourse.bass import ts
from concourse.tile import TileContext

D = 1024
BH = 1024
F32 = mybir.dt.float32
F32R = mybir.dt.float32r
K_TILES = D // 128   # 8 contraction tiles of 128
B_TILES = BH // 128  # 8 batch tiles of 128 per core
N_STEPS = 3 * K_TILES  # 24 (j,k) steps

Sin = mybir.ActivationFunctionType.Sin
Identity = mybir.ActivationFunctionType.Identity

N_WARM = 14  # warm-up matmuls covering the DMA prologue


def build_nc():
    nc = bacc.Bacc(None, target_bir_lowering=False, debug=False)
    xt = nc.declare_dram_parameter("xt", [3, D, BH], F32R, isOutput=False)
    wt = nc.declare_dram_parameter("wt", [3, D, D], F32R, isOutput=False)
    lf = nc.declare_dram_parameter("lf", [1, D], F32R, isOutput=False)
    sc = nc.declare_dram_parameter("sc", [1, 4], F32, isOutput=False)
    out = nc.declare_dram_parameter("out", [BH, D], F32, isOutput=True)

    with TileContext(nc) as tc:
        with (
            tc.tile_pool(name="wpool", bufs=N_STEPS) as wpool,
            tc.tile_pool(name="xpool", bufs=45) as xpool,
            tc.tile_pool(name="opool", bufs=2) as opool,
            tc.tile_pool(name="cpool", bufs=1) as cpool,
            tc.tile_pool(name="pspool", bufs=8, space="PSUM") as pspool,
        ):
            # --- prologue: memsets, warm-ups, oscillator bias ---
            ones = cpool.tile([1, 128], F32R, tag="ones", name="ones")
            nc.vector.memset(ones.bitcast(F32), 1.0)
            warm = cpool.tile([1, 512], F32R, tag="warm", name="warm")
            nc.vector.memset(warm.bitcast(F32), 0.0)

            # small config loads on the scalar queue (input stream owns sync)
            sc_sb = cpool.tile([1, 4], F32, tag="sc", name="sc_sb")
            nc.scalar.dma_start(out=sc_sb, in_=sc[:, :])
            brow = cpool.tile([1, D], F32R, tag="brow", name="brow")
            nc.scalar.dma_start(out=brow, in_=lf[:, :])

            # PE warm-up: keeps the HAM activity monitor busy from ~1us so
            # the clock steps to 2.4 GHz as early as possible.
            ps_warm = pspool.tile([128, 512], F32, tag="ps", name="ps_warm")
            for wi in range(N_WARM):
                nc.tensor.matmul(
                    ps_warm, lhsT=ones, rhs=warm,
                    start=(wi == 0), stop=(wi == N_WARM - 1),
                )

            # oscillator bias row: strength * (sin(t*lf) + sin(t*gf))
            # sc = [t, gf, strength, scratch]; gsin lands in sc[0, 3].
            nc.scalar.activation(brow, brow, Sin, scale=sc_sb[:, 0:1])
            nc.scalar.activation(
                sc_sb[:, 3:4], sc_sb[:, 1:2], Sin, scale=sc_sb[:, 0:1]
            )
            nc.scalar.activation(brow, brow, Identity, bias=sc_sb[:, 3:4])
            nc.scalar.activation(brow, brow, Identity, scale=sc_sb[:, 2:3])

            # broadcast bias row to [128, D] via ones-matmuls so the
            # vector engine can add it during every PSUM drain
            bcast = cpool.tile([128, D], F32, tag="bcast", name="bcast")
            ps_b = {}
            for o0 in range(2):
                ps_b[o0] = pspool.tile([128, 512], F32, tag="ps",
                                       name=f"ps_b{o0}")
                nc.tensor.matmul(
                    ps_b[o0], lhsT=ones, rhs=brow[:, ts(o0, 512)],
                    start=True, stop=True,
                )
            for o0 in range(2):
                nc.vector.tensor_copy(out=bcast[:, ts(o0, 512)], in_=ps_b[o0])

            # --- input kick stream (sync queue), consumption order ---
            # W tiles: single [128, D] 512KB kicks, except the first two
            # (j,k) steps which load in halves so the earliest matmuls
            # can start sooner (subtile deps).
            wtiles = []
            xg0 = []
            for i in range(N_STEPS):
                j, k = divmod(i, K_TILES)
                wti = wpool.tile([128, D], F32R, tag="wt", name=f"wt_{i}")
                if i < 2:
                    nc.sync.dma_start(
                        out=wti[:, ts(0, 512)], in_=wt[j, ts(k, 128), ts(0, 512)]
                    )
                    xti = xpool.tile([128, 512], F32R, tag="xt", name=f"x0_{i}")
                    nc.sync.dma_start(out=xti, in_=xt[j, ts(k, 128), ts(0, 512)])
                    nc.sync.dma_start(
                        out=wti[:, ts(1, 512)], in_=wt[j, ts(k, 128), ts(1, 512)]
                    )
                else:
                    nc.sync.dma_start(out=wti, in_=wt[j, ts(k, 128), :])
                    xti = xpool.tile([128, 512], F32R, tag="xt", name=f"x0_{i}")
                    nc.sync.dma_start(out=xti, in_=xt[j, ts(k, 128), ts(0, 512)])
                wtiles.append(wti)
                xg0.append(xti)
            xg1 = []
            for i in range(N_STEPS):
                j, k = divmod(i, K_TILES)
                xti = xpool.tile([128, 512], F32R, tag="xt", name=f"x1_{i}")
                nc.sync.dma_start(out=xti, in_=xt[j, ts(k, 128), ts(1, 512)])
                xg1.append(xti)

            def drain(ps_tile, bi, o0):
                ot = opool.tile([128, 512], F32, tag="ot",
                                name=f"ot_{bi}_{o0}")
                nc.vector.tensor_add(ot, ps_tile, bcast[:, ts(o0, 512)])
                nc.scalar.dma_start(
                    out=out[ts(bi, 128), ts(o0, 512)], in_=ot
                )

            # --- group 0: batch tiles 0..3, (j,k)-outer streaming ---
            ps0 = {}
            for bi in range(4):
                for o0 in range(2):
                    ps0[bi, o0] = pspool.tile([128, 512], F32, tag="ps",
                                              name=f"ps0_{bi}_{o0}")
            for i in range(N_STEPS - 1):
                # for the first (half-loaded W) steps, do all o0=0 before
                # o0=1 so the second W half has time to land
                if i < 2:
                    order = [(bi, o0) for o0 in range(2) for bi in range(4)]
                else:
                    order = [(bi, o0) for bi in range(4) for o0 in range(2)]
                for bi, o0 in order:
                    nc.tensor.matmul(
                        ps0[bi, o0],
                        lhsT=xg0[i][:, ts(bi, 128)],
                        rhs=wtiles[i][:, ts(o0, 512)],
                        start=(i == 0),
                        stop=False,
                    )
            # last step per-bi with immediate drains (staggers PSUM frees)
            i = N_STEPS - 1
            for bi in range(4):
                for o0 in range(2):
                    nc.tensor.matmul(
                        ps0[bi, o0],
                        lhsT=xg0[i][:, ts(bi, 128)],
                        rhs=wtiles[i][:, ts(o0, 512)],
                        start=False,
                        stop=True,
                    )
                for o0 in range(2):
                    drain(ps0[bi, o0], bi, o0)

            # --- group 1: batch tiles 4..7 ---
            # bi=4,5: (j,k)-outer (x stream may still be landing)
            ps1 = {}
            for bi in (4, 5):
                for o0 in range(2):
                    ps1[bi, o0] = pspool.tile([128, 512], F32, tag="ps",
                                              name=f"ps1_{bi}_{o0}")
            for i in range(N_STEPS - 1):
                for bi in (4, 5):
                    for o0 in range(2):
                        nc.tensor.matmul(
                            ps1[bi, o0],
                            lhsT=xg1[i][:, ts(bi - 4, 128)],
                            rhs=wtiles[i][:, ts(o0, 512)],
                            start=(i == 0),
                            stop=False,
                        )
            i = N_STEPS - 1
            for bi in (4, 5):
                for o0 in range(2):
                    nc.tensor.matmul(
                        ps1[bi, o0],
                        lhsT=xg1[i][:, ts(bi - 4, 128)],
                        rhs=wtiles[i][:, ts(o0, 512)],
                        start=False,
                        stop=True,
                    )
                for o0 in range(2):
                    drain(ps1[bi, o0], bi, o0)

            # bi=6,7: everything resident -> (j,k)-inner, drain per row
            # tile so the final output DMA overlaps remaining compute
            for bi in (6, 7):
                psb = {}
                for o0 in range(2):
                    psb[o0] = pspool.tile([128, 512], F32, tag="ps",
                                          name=f"ps1_{bi}_{o0}")
                for i in range(N_STEPS):
                    for o0 in range(2):
                        nc.tensor.matmul(
                            psb[o0],
                            lhsT=xg1[i][:, ts(bi - 4, 128)],
                            rhs=wtiles[i][:, ts(o0, 512)],
                            start=(i == 0),
                            stop=(i == N_STEPS - 1),
                        )
                for o0 in range(2):
                    drain(psb[o0], bi, o0)
    nc.finalize()
    return nc


def build_into(result):
    result["nc"] = build_nc()
'''

_builder_ns = {}
exec(compile(_BUILDER_SRC, _BUILDER_FILENAME, "exec"), _builder_ns)


def build_nc():
    """Build the (shared, SPMD) Bass program once.

    Runs in a thread whose entry point is the exec'd builder, so no frame
    with kernel.py's (location-dependent) path is on the stack while
    instructions capture debug info — the BIR stays byte-identical across
    directories and the NEFF compile cache stays warm."""
    result = {}
    t = threading.Thread(target=_builder_ns["build_into"], args=(result,))
    t.start()
    t.join()
    if "nc" not in result:
        # builder raised inside the thread; rebuild inline for a real trace
        return _builder_ns["build_nc"]()
    return result["nc"]


def make_in_maps(x, W, local_freq, global_freq, strength, current_clk):
    x = np.asarray(x, dtype=np.float32)
    W = np.asarray(W, dtype=np.float32)
    local_freq = np.asarray(local_freq, dtype=np.float32)
    global_freq = np.asarray(global_freq, dtype=np.float32)
    strength = np.asarray(strength, dtype=np.float32)
    clk = float(np.asarray(current_clk))
    t = 2.0 * math.pi * clk * 0.001

    in_maps = []
    for d in range(N_MOD):
        srcs = SRCS_OF[d]
        wt_d = np.ascontiguousarray(
            np.stack([W[PAIR_IDX[(s, d)]].T for s in srcs])
        )
        lf_d = np.ascontiguousarray(local_freq[d : d + 1])
        sc_d = np.array(
            [[t, float(global_freq[d]), float(strength[d]), 0.0]], dtype=np.float32
        )
        for h in range(2):
            xt_c = np.ascontiguousarray(
                np.stack([x[s, h * BH : (h + 1) * BH, :].T for s in srcs])
            )
            in_maps.append({"xt": xt_c, "wt": wt_d, "lf": lf_d, "sc": sc_d})
    return in_maps


def run(in_maps, trace=False, **kwargs):
    if "nc" not in _CACHED:
        _CACHED["nc"] = build_nc()
    res = run_bass_kernel_spmd(
        _CACHED["nc"], in_maps, core_ids=list(range(N_CORES)), trace=trace, **kwargs
    )
    return res


def kernel(x, W, local_freq, global_freq, strength, current_clk):
    in_maps = make_in_maps(x, W, local_freq, global_freq, strength, current_clk)
    res = run(in_maps)
    out = np.empty((N_MOD, B, D), dtype=np.float32)
    for d in range(N_MOD):
        for h in range(2):
            out[d, h * BH : (h + 1) * BH, :] = res.results[2 * d + h]["out"]
    return out


# revision 7
# speedup vs baseline: 1.0843x; 1.0843x over previous
"""Trainium2 Bass kernel for nn_AxonalConnections (gnn_message_passing).

Computes, for 4 modules with 12 directed pairs (s, d), s != d:
    out[d] = sum_{s != d} x[s] @ W[(s,d)].T
             + strength[d] * (sin(t*local_freq[d]) + sin(t*global_freq[d]))
with x: [4, 2048, 1024] f32, W: [12, 1024, 1024] f32, t = 2*pi*clk*1e-3.

Sharding over 8 NeuronCores: core c = 2*d + h handles destination module d
and batch half h (1024 rows).  Per core: 3 GEMMs [1024,1024]@[1024,1024]
accumulated in PSUM.  The oscillator bias row is broadcast to a [128,D]
SBUF tile once (ones-matmul) and added by the vector engine during the
PSUM->SBUF drain, so the PE only runs the 384 real matmuls.

Perf notes (v2):
- GEMM operands use float32r (TF32-class): 1 cycle/row on the PE for
  N>=256 — the PE stream floor is 384 x 512 rows ~ 82 us at 2.4 GHz and
  the kernel is PE-bound, so everything else is scheduled around keeping
  the PE stream gapless.
- DMA kick cost is ~0.6 us per dma_start on the issuing engine's queue:
  W tiles are fetched as single [128,1024] 512 KB kicks (24 instead of
  48) so the input stream is no longer kick-rate-bound (~380 GB/s in v1).
- Output drains go through the *scalar* engine's DMA queue; input loads
  own the sync queue.  This kills the head-of-line blocking that used to
  delay all 4 MiB of output writes to a serial tail after the last
  matmul.
- Batch group 1 computes its last two row-tiles (bi=6,7) with the
  (j,k)-sweep innermost and drains each row-tile immediately, so the
  final output DMA overlaps compute and the post-matmul tail is ~2 us.
- Dummy warm-up matmuls right after the entry barrier hold the PE's HAM
  activity monitor busy so the clock ramps to 2.4 GHz as early as
  possible.
- The Bass program is built by code exec'd under a fixed pseudo-filename
  so the BIR (which embeds source debug locations) is byte-identical no
  matter where kernel.py lives — keeping the NEFF compile cache warm
  across directories.

Host-side prep is limited to slicing/transposing inputs into the per-core
layouts (contraction dim on partitions) and computing the scalar t.
"""

import math
import sys
import threading

import numpy as np

sys.path.insert(0, "/opt/trn_rl_repo")

from concourse.bass_utils import run_bass_kernel_spmd  # noqa: E402

N_MOD = 4
B = 2048
D = 1024
BH = B // 2  # batch rows per core
N_CORES = 8

PAIRS = [(s, d) for s in range(N_MOD) for d in range(N_MOD) if s != d]
PAIR_IDX = {sd: i for i, sd in enumerate(PAIRS)}
SRCS_OF = {d: [s for s in range(N_MOD) if s != d] for d in range(N_MOD)}

_CACHED = {}

_BUILDER_FILENAME = "/bass_axonal_connections/builder.py"
_BUILDER_SRC = '''
import concourse.mybir as mybir
from concourse import bacc
from concourse.bass import ts
from concourse.tile import TileContext

D = 1024
BH = 1024
F32 = mybir.dt.float32
F32R = mybir.dt.float32r
K_TILES = D // 128   # 8 contraction tiles of 128
B_TILES = BH // 128  # 8 batch tiles of 128 per core
N_STEPS = 3 * K_TILES  # 24 (j,k) steps

Sin = mybir.ActivationFunctionType.Sin
Identity = mybir.ActivationFunctionType.Identity

N_WARM = 26  # warm-up matmuls covering the DMA prologue
TAIL = 4     # trailing (j,k) steps swept per-bi so drains stagger


def build_nc():
    nc = bacc.Bacc(None, target_bir_lowering=False, debug=False)
    xt = nc.declare_dram_parameter("xt", [3, D, BH], F32R, isOutput=False)
    wt = nc.declare_dram_parameter("wt", [3, D, D], F32R, isOutput=False)
    lf = nc.declare_dram_parameter("lf", [1, D], F32R, isOutput=False)
    sc = nc.declare_dram_parameter("sc", [1, 4], F32, isOutput=False)
    out = nc.declare_dram_parameter("out", [BH, D], F32, isOutput=True)

    with TileContext(nc) as tc:
        with (
            tc.tile_pool(name="wpool", bufs=N_STEPS) as wpool,
            tc.tile_pool(name="xpool", bufs=39) as xpool,
            tc.tile_pool(name="opool", bufs=8) as opool,
            tc.tile_pool(name="cpool", bufs=1) as cpool,
            tc.tile_pool(name="pspool", bufs=8, space="PSUM") as pspool,
        ):
            # --- prologue: memsets, warm-ups, oscillator bias ---
            ones = cpool.tile([1, 128], F32R, tag="ones", name="ones")
            nc.vector.memset(ones.bitcast(F32), 1.0)
            warm = cpool.tile([1, 512], F32R, tag="warm", name="warm")
            nc.vector.memset(warm.bitcast(F32), 0.0)

            # small config loads on the scalar queue (input stream owns sync)
            sc_sb = cpool.tile([1, 4], F32, tag="sc", name="sc_sb")
            nc.scalar.dma_start(out=sc_sb, in_=sc[:, :])
            brow = cpool.tile([1, D], F32R, tag="brow", name="brow")
            nc.scalar.dma_start(out=brow, in_=lf[:, :])

            # PE warm-up: keeps the HAM activity monitor busy from ~1us so
            # the clock steps to 2.4 GHz as early as possible.
            ps_warm = pspool.tile([128, 512], F32, tag="ps", name="ps_warm")
            for wi in range(N_WARM):
                nc.tensor.matmul(
                    ps_warm, lhsT=ones, rhs=warm,
                    start=(wi == 0), stop=(wi == N_WARM - 1),
                )

            # oscillator bias row: strength * (sin(t*lf) + sin(t*gf))
            # sc = [t, gf, strength, scratch]; gsin lands in sc[0, 3].
            nc.scalar.activation(brow, brow, Sin, scale=sc_sb[:, 0:1])
            nc.scalar.activation(
                sc_sb[:, 3:4], sc_sb[:, 1:2], Sin, scale=sc_sb[:, 0:1]
            )
            nc.scalar.activation(brow, brow, Identity, bias=sc_sb[:, 3:4])
            nc.scalar.activation(brow, brow, Identity, scale=sc_sb[:, 2:3])

            # broadcast bias row to [128, D] via ones-matmuls so the
            # vector engine can add it during every PSUM drain
            bcast = cpool.tile([128, D], F32, tag="bcast", name="bcast")
            ps_b = {}
            for o0 in range(2):
                ps_b[o0] = pspool.tile([128, 512], F32, tag="ps",
                                       name=f"ps_b{o0}")
                nc.tensor.matmul(
                    ps_b[o0], lhsT=ones, rhs=brow[:, ts(o0, 512)],
                    start=True, stop=True,
                )
            for o0 in range(2):
                nc.vector.tensor_copy(out=bcast[:, ts(o0, 512)], in_=ps_b[o0])

            # --- input kick stream (sync queue), consumption order ---
            # W tiles: single [128, D] 512KB kicks, except the first two
            # (j,k) steps which load in halves so the earliest matmuls
            # can start sooner (subtile deps).
            wtiles = []
            xg0 = []
            for i in range(N_STEPS):
                j, k = divmod(i, K_TILES)
                wti = wpool.tile([128, D], F32R, tag="wt", name=f"wt_{i}")
                if i < 2:
                    nc.sync.dma_start(
                        out=wti[:, ts(0, 512)], in_=wt[j, ts(k, 128), ts(0, 512)]
                    )
                    xti = xpool.tile([128, 512], F32R, tag="xt", name=f"x0_{i}")
                    nc.sync.dma_start(out=xti, in_=xt[j, ts(k, 128), ts(0, 512)])
                    nc.sync.dma_start(
                        out=wti[:, ts(1, 512)], in_=wt[j, ts(k, 128), ts(1, 512)]
                    )
                else:
                    nc.sync.dma_start(out=wti, in_=wt[j, ts(k, 128), :])
                    xti = xpool.tile([128, 512], F32R, tag="xt", name=f"x0_{i}")
                    nc.sync.dma_start(out=xti, in_=xt[j, ts(k, 128), ts(0, 512)])
                wtiles.append(wti)
                xg0.append(xti)
            xg1 = []
            for i in range(N_STEPS):
                j, k = divmod(i, K_TILES)
                xti = xpool.tile([128, 512], F32R, tag="xt", name=f"x1_{i}")
                nc.sync.dma_start(out=xti, in_=xt[j, ts(k, 128), ts(1, 512)])
                xg1.append(xti)

            def drain(ps_tile, bi, o0):
                ot = opool.tile([128, 512], F32, tag="ot",
                                name=f"ot_{bi}_{o0}")
                nc.vector.tensor_add(ot, ps_tile, bcast[:, ts(o0, 512)])
                nc.scalar.dma_start(
                    out=out[ts(bi, 128), ts(o0, 512)], in_=ot
                )

            # --- group 0: batch tiles 0..3, (j,k)-outer streaming ---
            ps0 = {}
            for bi in range(4):
                for o0 in range(2):
                    ps0[bi, o0] = pspool.tile([128, 512], F32, tag="ps",
                                              name=f"ps0_{bi}_{o0}")
            for i in range(N_STEPS - TAIL):
                # for the first (half-loaded W) steps, do all o0=0 before
                # o0=1 so the second W half has time to land
                if i < 2:
                    order = [(bi, o0) for o0 in range(2) for bi in range(4)]
                else:
                    order = [(bi, o0) for bi in range(4) for o0 in range(2)]
                for bi, o0 in order:
                    nc.tensor.matmul(
                        ps0[bi, o0],
                        lhsT=xg0[i][:, ts(bi, 128)],
                        rhs=wtiles[i][:, ts(o0, 512)],
                        start=(i == 0),
                        stop=False,
                    )
            # trailing steps per-bi with immediate drains: PSUM banks free
            # progressively so group 1 starts without stalling
            for bi in range(4):
                for i in range(N_STEPS - TAIL, N_STEPS):
                    for o0 in range(2):
                        nc.tensor.matmul(
                            ps0[bi, o0],
                            lhsT=xg0[i][:, ts(bi, 128)],
                            rhs=wtiles[i][:, ts(o0, 512)],
                            start=False,
                            stop=(i == N_STEPS - 1),
                        )
                for o0 in range(2):
                    drain(ps0[bi, o0], bi, o0)

            # --- group 1: batch tiles 4..7 ---
            # bi=4,5: (j,k)-outer (x stream may still be landing)
            ps1 = {}
            for bi in (4, 5):
                for o0 in range(2):
                    ps1[bi, o0] = pspool.tile([128, 512], F32, tag="ps",
                                              name=f"ps1_{bi}_{o0}")
            for i in range(N_STEPS - TAIL):
                for bi in (4, 5):
                    for o0 in range(2):
                        nc.tensor.matmul(
                            ps1[bi, o0],
                            lhsT=xg1[i][:, ts(bi - 4, 128)],
                            rhs=wtiles[i][:, ts(o0, 512)],
                            start=(i == 0),
                            stop=False,
                        )
            for bi in (4, 5):
                for i in range(N_STEPS - TAIL, N_STEPS):
                    for o0 in range(2):
                        nc.tensor.matmul(
                            ps1[bi, o0],
                            lhsT=xg1[i][:, ts(bi - 4, 128)],
                            rhs=wtiles[i][:, ts(o0, 512)],
                            start=False,
                            stop=(i == N_STEPS - 1),
                        )
                for o0 in range(2):
                    drain(ps1[bi, o0], bi, o0)

            # bi=6,7: everything resident -> (j,k)-inner, drain per row
            # tile so the final output DMA overlaps remaining compute
            for bi in (6, 7):
                psb = {}
                for o0 in range(2):
                    psb[o0] = pspool.tile([128, 512], F32, tag="ps",
                                          name=f"ps1_{bi}_{o0}")
                for i in range(N_STEPS):
                    for o0 in range(2):
                        nc.tensor.matmul(
                            psb[o0],
                            lhsT=xg1[i][:, ts(bi - 4, 128)],
                            rhs=wtiles[i][:, ts(o0, 512)],
                            start=(i == 0),
                            stop=(i == N_STEPS - 1),
                        )
                for o0 in range(2):
                    drain(psb[o0], bi, o0)
    nc.finalize()
    return nc


def build_into(result):
    result["nc"] = build_nc()
'''

_builder_ns = {}
exec(compile(_BUILDER_SRC, _BUILDER_FILENAME, "exec"), _builder_ns)


def build_nc():
    """Build the (shared, SPMD) Bass program once.

    Runs in a thread whose entry point is the exec'd builder, so no frame
    with kernel.py's (location-dependent) path is on the stack while
    instructions capture debug info — the BIR stays byte-identical across
    directories and the NEFF compile cache stays warm."""
    result = {}
    t = threading.Thread(target=_builder_ns["build_into"], args=(result,))
    t.start()
    t.join()
    if "nc" not in result:
        # builder raised inside the thread; rebuild inline for a real trace
        return _builder_ns["build_nc"]()
    return result["nc"]


def make_in_maps(x, W, local_freq, global_freq, strength, current_clk):
    x = np.asarray(x, dtype=np.float32)
    W = np.asarray(W, dtype=np.float32)
    local_freq = np.asarray(local_freq, dtype=np.float32)
    global_freq = np.asarray(global_freq, dtype=np.float32)
    strength = np.asarray(strength, dtype=np.float32)
    clk = float(np.asarray(current_clk))
    t = 2.0 * math.pi * clk * 0.001

    in_maps = []
    for d in range(N_MOD):
        srcs = SRCS_OF[d]
        wt_d = np.ascontiguousarray(
            np.stack([W[PAIR_IDX[(s, d)]].T for s in srcs])
        )
        lf_d = np.ascontiguousarray(local_freq[d : d + 1])
        sc_d = np.array(
            [[t, float(global_freq[d]), float(strength[d]), 0.0]], dtype=np.float32
        )
        for h in range(2):
            xt_c = np.ascontiguousarray(
                np.stack([x[s, h * BH : (h + 1) * BH, :].T for s in srcs])
            )
            in_maps.append({"xt": xt_c, "wt": wt_d, "lf": lf_d, "sc": sc_d})
    return in_maps


def run(in_maps, trace=False, **kwargs):
    if "nc" not in _CACHED:
        _CACHED["nc"] = build_nc()
    res = run_bass_kernel_spmd(
        _CACHED["nc"], in_maps, core_ids=list(range(N_CORES)), trace=trace, **kwargs
    )
    return res


def kernel(x, W, local_freq, global_freq, strength, current_clk):
    in_maps = make_in_maps(x, W, local_freq, global_freq, strength, current_clk)
    res = run(in_maps)
    out = np.empty((N_MOD, B, D), dtype=np.float32)
    for d in range(N_MOD):
        for h in range(2):
            out[d, h * BH : (h + 1) * BH, :] = res.results[2 * d + h]["out"]
    return out


# revision 9
# speedup vs baseline: 1.1021x; 1.0165x over previous
"""Trainium2 Bass kernel for nn_AxonalConnections (gnn_message_passing).

Computes, for 4 modules with 12 directed pairs (s, d), s != d:
    out[d] = sum_{s != d} x[s] @ W[(s,d)].T
             + strength[d] * (sin(t*local_freq[d]) + sin(t*global_freq[d]))
with x: [4, 2048, 1024] f32, W: [12, 1024, 1024] f32, t = 2*pi*clk*1e-3.

Sharding over 8 NeuronCores: core c = 2*d + h handles destination module d
and batch half h (1024 rows).  Per core: 3 GEMMs [1024,1024]@[1024,1024]
accumulated in PSUM.  The oscillator bias row is broadcast to a [128,D]
SBUF tile once (ones-matmul) and added by the vector engine during the
PSUM->SBUF drain, so the PE only runs the 384 real matmuls.

Perf notes (v2):
- GEMM operands use float32r (TF32-class): 1 cycle/row on the PE for
  N>=256 — the PE stream floor is 384 x 512 rows ~ 82 us at 2.4 GHz and
  the kernel is PE-bound, so everything else is scheduled around keeping
  the PE stream gapless.
- DMA kick cost is ~0.6 us per dma_start on the issuing engine's queue:
  W tiles are fetched as single [128,1024] 512 KB kicks (24 instead of
  48) so the input stream is no longer kick-rate-bound (~380 GB/s in v1).
- Output drains go through the *scalar* engine's DMA queue; input loads
  own the sync queue.  This kills the head-of-line blocking that used to
  delay all 4 MiB of output writes to a serial tail after the last
  matmul.
- Batch group 1 computes its last two row-tiles (bi=6,7) with the
  (j,k)-sweep innermost and drains each row-tile immediately, so the
  final output DMA overlaps compute and the post-matmul tail is ~2 us.
- Dummy warm-up matmuls right after the entry barrier hold the PE's HAM
  activity monitor busy so the clock ramps to 2.4 GHz as early as
  possible.
- The Bass program is built by code exec'd under a fixed pseudo-filename
  so the BIR (which embeds source debug locations) is byte-identical no
  matter where kernel.py lives — keeping the NEFF compile cache warm
  across directories.

Host-side prep is limited to slicing/transposing inputs into the per-core
layouts (contraction dim on partitions) and computing the scalar t.
"""

import math
import sys
import threading

import numpy as np

sys.path.insert(0, "/opt/trn_rl_repo")

from concourse.bass_utils import run_bass_kernel_spmd  # noqa: E402

N_MOD = 4
B = 2048
D = 1024
BH = B // 2  # batch rows per core
N_CORES = 8

PAIRS = [(s, d) for s in range(N_MOD) for d in range(N_MOD) if s != d]
PAIR_IDX = {sd: i for i, sd in enumerate(PAIRS)}
SRCS_OF = {d: [s for s in range(N_MOD) if s != d] for d in range(N_MOD)}

_CACHED = {}

_BUILDER_FILENAME = "/bass_axonal_connections/builder.py"
_BUILDER_SRC = '''
import concourse.mybir as mybir
from concourse import bacc
from concourse.bass import ts
from concourse.tile import TileContext

D = 1024
BH = 1024
F32 = mybir.dt.float32
F32R = mybir.dt.float32r
K_TILES = D // 128   # 8 contraction tiles of 128
B_TILES = BH // 128  # 8 batch tiles of 128 per core
N_STEPS = 3 * K_TILES  # 24 (j,k) steps

Sin = mybir.ActivationFunctionType.Sin
Identity = mybir.ActivationFunctionType.Identity

N_WARM = 6   # warm-up matmuls covering the DMA prologue (~427ns each cold)
TAIL = 4     # trailing (j,k) steps swept per-bi so drains stagger


def build_nc():
    nc = bacc.Bacc(None, target_bir_lowering=False, debug=False)
    xt = nc.declare_dram_parameter("xt", [3, D, BH], F32R, isOutput=False)
    wt = nc.declare_dram_parameter("wt", [3, D, D], F32R, isOutput=False)
    lf = nc.declare_dram_parameter("lf", [1, D], F32R, isOutput=False)
    sc = nc.declare_dram_parameter("sc", [1, 4], F32, isOutput=False)
    out = nc.declare_dram_parameter("out", [BH, D], F32, isOutput=True)

    with TileContext(nc) as tc:
        with (
            tc.tile_pool(name="wpool", bufs=N_STEPS) as wpool,
            tc.tile_pool(name="xpool", bufs=39) as xpool,
            tc.tile_pool(name="opool", bufs=8) as opool,
            tc.tile_pool(name="cpool", bufs=1) as cpool,
            tc.tile_pool(name="pspool", bufs=8, space="PSUM") as pspool,
        ):
            # --- prologue: memsets, warm-ups, oscillator bias ---
            ones = cpool.tile([1, 128], F32R, tag="ones", name="ones")
            nc.vector.memset(ones.bitcast(F32), 1.0)
            warm = cpool.tile([1, 512], F32R, tag="warm", name="warm")
            nc.vector.memset(warm.bitcast(F32), 0.0)

            # small config loads on the scalar queue (input stream owns sync)
            sc_sb = cpool.tile([1, 4], F32, tag="sc", name="sc_sb")
            nc.scalar.dma_start(out=sc_sb, in_=sc[:, :])
            brow = cpool.tile([1, D], F32R, tag="brow", name="brow")
            nc.scalar.dma_start(out=brow, in_=lf[:, :])

            # PE warm-up: keeps the HAM activity monitor busy from ~1us so
            # the clock steps to 2.4 GHz as early as possible.
            ps_warm = pspool.tile([128, 512], F32, tag="ps", name="ps_warm")
            for wi in range(N_WARM):
                nc.tensor.matmul(
                    ps_warm, lhsT=ones, rhs=warm,
                    start=(wi == 0), stop=(wi == N_WARM - 1),
                )

            # oscillator bias row: strength * (sin(t*lf) + sin(t*gf))
            # sc = [t, gf, strength, scratch]; gsin lands in sc[0, 3].
            nc.scalar.activation(brow, brow, Sin, scale=sc_sb[:, 0:1])
            nc.scalar.activation(
                sc_sb[:, 3:4], sc_sb[:, 1:2], Sin, scale=sc_sb[:, 0:1]
            )
            nc.scalar.activation(brow, brow, Identity, bias=sc_sb[:, 3:4])
            nc.scalar.activation(brow, brow, Identity, scale=sc_sb[:, 2:3])

            # broadcast bias row to [128, D] via ones-matmuls so the
            # vector engine can add it during every PSUM drain
            bcast = cpool.tile([128, D], F32, tag="bcast", name="bcast")
            ps_b = {}
            for o0 in range(2):
                ps_b[o0] = pspool.tile([128, 512], F32, tag="ps",
                                       name=f"ps_b{o0}")
                nc.tensor.matmul(
                    ps_b[o0], lhsT=ones, rhs=brow[:, ts(o0, 512)],
                    start=True, stop=True,
                )
            for o0 in range(2):
                nc.vector.tensor_copy(out=bcast[:, ts(o0, 512)], in_=ps_b[o0])

            # --- input kick stream (sync queue), consumption order ---
            # W tiles: single [128, D] 512KB kicks, except the first two
            # (j,k) steps which load in halves so the earliest matmuls
            # can start sooner (subtile deps).
            wtiles = []
            xg0 = []
            for i in range(N_STEPS):
                j, k = divmod(i, K_TILES)
                wti = wpool.tile([128, D], F32R, tag="wt", name=f"wt_{i}")
                if i < 2:
                    nc.sync.dma_start(
                        out=wti[:, ts(0, 512)], in_=wt[j, ts(k, 128), ts(0, 512)]
                    )
                    xti = xpool.tile([128, 512], F32R, tag="xt", name=f"x0_{i}")
                    nc.sync.dma_start(out=xti, in_=xt[j, ts(k, 128), ts(0, 512)])
                    nc.sync.dma_start(
                        out=wti[:, ts(1, 512)], in_=wt[j, ts(k, 128), ts(1, 512)]
                    )
                else:
                    nc.sync.dma_start(out=wti, in_=wt[j, ts(k, 128), :])
                    xti = xpool.tile([128, 512], F32R, tag="xt", name=f"x0_{i}")
                    nc.sync.dma_start(out=xti, in_=xt[j, ts(k, 128), ts(0, 512)])
                wtiles.append(wti)
                xg0.append(xti)
            xg1 = []
            for i in range(N_STEPS):
                j, k = divmod(i, K_TILES)
                xti = xpool.tile([128, 512], F32R, tag="xt", name=f"x1_{i}")
                nc.sync.dma_start(out=xti, in_=xt[j, ts(k, 128), ts(1, 512)])
                xg1.append(xti)

            def drain(ps_tile, bi, o0):
                ot = opool.tile([128, 512], F32, tag="ot",
                                name=f"ot_{bi}_{o0}")
                nc.vector.tensor_add(ot, ps_tile, bcast[:, ts(o0, 512)])
                nc.scalar.dma_start(
                    out=out[ts(bi, 128), ts(o0, 512)], in_=ot
                )

            # --- group 0: batch tiles 0..3, (j,k)-outer streaming ---
            ps0 = {}
            for bi in range(4):
                for o0 in range(2):
                    ps0[bi, o0] = pspool.tile([128, 512], F32, tag="ps",
                                              name=f"ps0_{bi}_{o0}")
            for i in range(N_STEPS - TAIL):
                # for the first (half-loaded W) steps, do all o0=0 before
                # o0=1 so the second W half has time to land
                if i < 2:
                    order = [(bi, o0) for o0 in range(2) for bi in range(4)]
                else:
                    order = [(bi, o0) for bi in range(4) for o0 in range(2)]
                for bi, o0 in order:
                    nc.tensor.matmul(
                        ps0[bi, o0],
                        lhsT=xg0[i][:, ts(bi, 128)],
                        rhs=wtiles[i][:, ts(o0, 512)],
                        start=(i == 0),
                        stop=False,
                    )
            # trailing steps per-bi with immediate drains: PSUM banks free
            # progressively so group 1 starts without stalling
            for bi in range(4):
                for i in range(N_STEPS - TAIL, N_STEPS):
                    for o0 in range(2):
                        nc.tensor.matmul(
                            ps0[bi, o0],
                            lhsT=xg0[i][:, ts(bi, 128)],
                            rhs=wtiles[i][:, ts(o0, 512)],
                            start=False,
                            stop=(i == N_STEPS - 1),
                        )
                for o0 in range(2):
                    drain(ps0[bi, o0], bi, o0)

            # --- group 1: batch tiles 4..7 ---
            # bi=4,5: (j,k)-outer (x stream may still be landing)
            ps1 = {}
            for bi in (4, 5):
                for o0 in range(2):
                    ps1[bi, o0] = pspool.tile([128, 512], F32, tag="ps",
                                              name=f"ps1_{bi}_{o0}")
            for i in range(N_STEPS - TAIL):
                for bi in (4, 5):
                    for o0 in range(2):
                        nc.tensor.matmul(
                            ps1[bi, o0],
                            lhsT=xg1[i][:, ts(bi - 4, 128)],
                            rhs=wtiles[i][:, ts(o0, 512)],
                            start=(i == 0),
                            stop=False,
                        )
            for bi in (4, 5):
                for i in range(N_STEPS - TAIL, N_STEPS):
                    for o0 in range(2):
                        nc.tensor.matmul(
                            ps1[bi, o0],
                            lhsT=xg1[i][:, ts(bi - 4, 128)],
                            rhs=wtiles[i][:, ts(o0, 512)],
                            start=False,
                            stop=(i == N_STEPS - 1),
                        )
                for o0 in range(2):
                    drain(ps1[bi, o0], bi, o0)

            # bi=6,7: everything resident -> (j,k)-inner, drain per row
            # tile so the final output DMA overlaps remaining compute
            for bi in (6, 7):
                psb = {}
                for o0 in range(2):
                    psb[o0] = pspool.tile([128, 512], F32, tag="ps",
                                          name=f"ps1_{bi}_{o0}")
                for i in range(N_STEPS - 1):
                    for o0 in range(2):
                        nc.tensor.matmul(
                            psb[o0],
                            lhsT=xg1[i][:, ts(bi - 4, 128)],
                            rhs=wtiles[i][:, ts(o0, 512)],
                            start=(i == 0),
                            stop=False,
                        )
                # last step: drain o0 while the o1 matmul still runs
                i = N_STEPS - 1
                for o0 in range(2):
                    nc.tensor.matmul(
                        psb[o0],
                        lhsT=xg1[i][:, ts(bi - 4, 128)],
                        rhs=wtiles[i][:, ts(o0, 512)],
                        start=False,
                        stop=True,
                    )
                    drain(psb[o0], bi, o0)
    nc.finalize()
    return nc


def build_into(result):
    result["nc"] = build_nc()
'''

_builder_ns = {}
exec(compile(_BUILDER_SRC, _BUILDER_FILENAME, "exec"), _builder_ns)


def build_nc():
    """Build the (shared, SPMD) Bass program once.

    Runs in a thread whose entry point is the exec'd builder, so no frame
    with kernel.py's (location-dependent) path is on the stack while
    instructions capture debug info — the BIR stays byte-identical across
    directories and the NEFF compile cache stays warm."""
    result = {}
    t = threading.Thread(target=_builder_ns["build_into"], args=(result,))
    t.start()
    t.join()
    if "nc" not in result:
        # builder raised inside the thread; rebuild inline for a real trace
        return _builder_ns["build_nc"]()
    return result["nc"]


def make_in_maps(x, W, local_freq, global_freq, strength, current_clk):
    x = np.asarray(x, dtype=np.float32)
    W = np.asarray(W, dtype=np.float32)
    local_freq = np.asarray(local_freq, dtype=np.float32)
    global_freq = np.asarray(global_freq, dtype=np.float32)
    strength = np.asarray(strength, dtype=np.float32)
    clk = float(np.asarray(current_clk))
    t = 2.0 * math.pi * clk * 0.001

    in_maps = []
    for d in range(N_MOD):
        srcs = SRCS_OF[d]
        wt_d = np.ascontiguousarray(
            np.stack([W[PAIR_IDX[(s, d)]].T for s in srcs])
        )
        lf_d = np.ascontiguousarray(local_freq[d : d + 1])
        sc_d = np.array(
            [[t, float(global_freq[d]), float(strength[d]), 0.0]], dtype=np.float32
        )
        for h in range(2):
            xt_c = np.ascontiguousarray(
                np.stack([x[s, h * BH : (h + 1) * BH, :].T for s in srcs])
            )
            in_maps.append({"xt": xt_c, "wt": wt_d, "lf": lf_d, "sc": sc_d})
    return in_maps


def run(in_maps, trace=False, **kwargs):
    if "nc" not in _CACHED:
        _CACHED["nc"] = build_nc()
    res = run_bass_kernel_spmd(
        _CACHED["nc"], in_maps, core_ids=list(range(N_CORES)), trace=trace, **kwargs
    )
    return res


def kernel(x, W, local_freq, global_freq, strength, current_clk):
    in_maps = make_in_maps(x, W, local_freq, global_freq, strength, current_clk)
    res = run(in_maps)
    out = np.empty((N_MOD, B, D), dtype=np.float32)
    for d in range(N_MOD):
        for h in range(2):
            out[d, h * BH : (h + 1) * BH, :] = res.results[2 * d + h]["out"]
    return out
